# revision 12
# baseline (speedup 1.0000x reference)
"""Multi-head attention (b=2, l=2048, d_model=1024, h=16) on 8 trn2 NeuronCores.

Sharding: tensor-parallel over heads. Each core owns 2 heads: it computes the
QKV projections for its 128 channels (transposed layout), attention for its
heads, and a rank-128 partial of the output projection. The host sums the 8
partials and adds b_o (the tensor-parallel all-reduce, done at gather time).

v2 design (vs the filler-matmul baseline): bf16 data path, no fillers.
  warmup:  identity matmul burst (clock ramp) + dummy exp (ACT table preload).
  A0:      QKV projections for batch-0 chunks only (bf16 x streamed in 512-tok
           chunks); V re-transposed to natural layout tiles with a ones column.
  B:       per (batch, head, 1024-q-chunk, k-tile): scoresT = KT@QT (bf16),
           expT on ACT (the pacer, f32 psum -> bf16), PV accumulates
           [V_h|1].T @ expT -> psum [65, 1024]. A side-work queue fills the
           PE's ACT-slack with real work instead of fillers: batch-1's QKV
           chunks during batch-0's windows, then batch-0's output projection
           during batch-1's windows.
  norm:    Z rows -> reciprocal (DVE) -> rank-1 PE broadcast [1/Z broadcast
           over channel partitions] -> one DVE multiply normalizes attnU, so
           the output projection is a single full-128-contraction matmul per
           tile (half the PE columns + LDWEIGHTS of the deferred-scale split).
  C:       out[tok, :] = attnN.T @ Wo per 128-token tile; psum evacuated to
           bf16 staging (DVE/ACT) and DMA'd out.
"""
import sys
import types

import numpy as np

D_MODEL = 1024
H = 16
DH = 64
B = 2
L = 2048
BL = B * L            # 4096 tokens
NCORES = 8
NKT = D_MODEL // 128  # 8 feature tiles
TCH = 512             # phase-A token chunk
NCH = BL // TCH       # 8 chunks
QC = 1024             # phase-B q chunk
NQC = L // QC         # 2 per batch
NKB = L // 128        # 16 k-tiles per batch
VSTRIDE = 2 * (DH + 1)  # per-k-tile Vaug columns: [V_h0 | 1 | V_h1 | 1]


def _register_ntff_hook():
    """Install the axon NTFF profiling hook module if the image lacks it.

    Harmless if never used; required for run_bass_kernel_spmd(trace=True)."""
    if "antenv.axon_hooks" in sys.modules:
        return
    try:
        import antenv
        mod = types.ModuleType("antenv.axon_hooks")
        holder = {}
        mod.set_axon_ntff_profile_hook = lambda h: holder.__setitem__("h", h)
        mod.get_axon_ntff_profile_hook = lambda: holder.get("h")
        sys.modules["antenv.axon_hooks"] = mod
        antenv.axon_hooks = mod
        from trn_agent_boot.trn_boot import _ntff_profile_via_ctypes
        mod.set_axon_ntff_profile_hook(
            _ntff_profile_via_ctypes("/opt/axon/libaxon_pjrt.so")
        )
    except Exception:
        pass


_NC_CACHE = {}


def _build():
    if "nc" in _NC_CACHE:
        return _NC_CACHE["nc"]
    import concourse.bacc as bacc
    import concourse.tile as tile
    import concourse.mybir as mybir

    F32 = mybir.dt.float32
    F32R = mybir.dt.float32r
    BF16 = mybir.dt.bfloat16
    AF = mybir.ActivationFunctionType
    ALU = mybir.AluOpType

    nc = bacc.Bacc("TRN2", target_bir_lowering=False, debug=False)

    xT_d = nc.dram_tensor("xT", [D_MODEL, BL], BF16, kind="ExternalInput").ap()
    wq_d = nc.dram_tensor("wq", [128, NKT * 128], BF16, kind="ExternalInput").ap()
    wk_d = nc.dram_tensor("wk", [128, NKT * 128], BF16, kind="ExternalInput").ap()
    wv_d = nc.dram_tensor("wv", [128, NKT * 128], BF16, kind="ExternalInput").ap()
    bq_d = nc.dram_tensor("bq", [128, 1], F32, kind="ExternalInput").ap()
    bk_d = nc.dram_tensor("bk", [128, 1], F32, kind="ExternalInput").ap()
    bv_d = nc.dram_tensor("bv", [128, 1], F32, kind="ExternalInput").ap()
    wo_d = nc.dram_tensor("wo", [128, D_MODEL], BF16, kind="ExternalInput").ap()
    id_d = nc.dram_tensor("ident", [128, 128], BF16, kind="ExternalInput").ap()
    out_d = nc.dram_tensor("out", [BL, D_MODEL], BF16, kind="ExternalOutput").ap()

    with tile.TileContext(nc) as tc:
        with (
            tc.tile_pool(name="weights", bufs=1) as wpool,
            tc.tile_pool(name="persist", bufs=1) as ppool,
        ):
            id_t = wpool.tile([128, 128], BF16, tag="ident")
            nc.gpsimd.dma_start(id_t[:], id_d)
            wq_t = wpool.tile([128, NKT * 128], BF16, tag="wq")
            wk_t = wpool.tile([128, NKT * 128], BF16, tag="wk")
            wv_t = wpool.tile([128, NKT * 128], BF16, tag="wv")
            bq_t = wpool.tile([128, 1], F32, tag="bq")
            bk_t = wpool.tile([128, 1], F32, tag="bk")
            bv_t = wpool.tile([128, 1], F32, tag="bv")
            wo_t = wpool.tile([128, D_MODEL], BF16, tag="wo")
            for t, d in ((wq_t, wq_d), (wk_t, wk_d), (wv_t, wv_d),
                         (bq_t, bq_d), (bk_t, bk_d), (bv_t, bv_d),
                         (wo_t, wo_d)):
                nc.gpsimd.dma_start(t[:], d)

            QT = ppool.tile([128, BL], BF16, tag="QT")
            KT = ppool.tile([128, BL], BF16, tag="KT")
            VT = ppool.tile([128, BL], BF16, tag="VT")
            Vaug = ppool.tile([128, (BL // 128) * VSTRIDE], BF16, tag="Vaug")
            attnU = [ppool.tile([128, L], BF16, tag=f"attnU{b}",
                                name=f"attnU{b}") for b in range(B)]
            # softmax denominators Z: h0 at partition 0, h1 at
            # partition 32 (engine writes need 32-aligned base partitions)
            zb = [ppool.tile([33, L], BF16, tag=f"zb{b}",
                             name=f"zb{b}") for b in range(B)]
            # head-half selector: rows 0 / 32 pick head halves, rest zero
            sel_t = ppool.tile([33, 128], BF16, tag="sel")
            scr = ppool.tile([1, 32], F32, tag="scr")

            # packed pair of bf16 1.0s viewed as f32
            ONE2 = float(np.frombuffer(
                np.uint32(0x3F803F80).tobytes(), dtype=np.float32)[0])
            nc.vector.memset(Vaug[:].bitcast(F32), ONE2)
            nc.vector.memset(sel_t[:].bitcast(F32), 0.0)
            nc.vector.memset(sel_t[:].bitcast(F32)[0:1, 0:32], ONE2)
            nc.vector.memset(sel_t[:].bitcast(F32)[32:33, 32:64], ONE2)
            for b in range(B):
                nc.vector.memset(zb[b][:].bitcast(F32), ONE2)

            with tc.tile_pool(name="psX", bufs=2, space="PSUM") as psX:
                # ---- warmup: lift clock gate + preload exp table ----
                wu = psX.tile([128, 512], F32, tag="x")
                for i in range(40):
                    nc.tensor.matmul(wu[:, 0:128], id_t[:], id_t[:],
                                     start=(i == 0), stop=(i == 39))
                nc.scalar.activation(scr[:], wu[0:1, 0:32], AF.Exp)

                chunk_xt = {}

                def emit_chunk_dma(c, xpool):
                    xt = xpool.tile([128, NKT, TCH], BF16, tag="xchunk",
                                    name=f"xt{c}")
                    sl = slice(c * TCH, (c + 1) * TCH)
                    for kt in range(NKT):
                        nc.sync.dma_start(
                            xt[:, kt, :], xT_d[kt * 128:(kt + 1) * 128, sl]
                        )
                    chunk_xt[c] = xt

                def emit_proj(c, w_t, b_t, dst):
                    xt = chunk_xt[c]
                    ps = psX.tile([128, TCH], F32, tag="x", name="projps")
                    for kt in range(NKT):
                        nc.tensor.matmul(
                            ps[:], w_t[:, kt * 128:(kt + 1) * 128],
                            xt[:, kt, :],
                            start=(kt == 0), stop=(kt == NKT - 1),
                        )
                    sl = slice(c * TCH, (c + 1) * TCH)
                    nc.vector.tensor_scalar_add(dst[:, sl], ps[:], b_t[:, 0:1])

                def emit_tr(c):
                    # natural-layout V (with ones cols) for this chunk's tiles
                    for g in range(c * (TCH // 128), (c + 1) * (TCH // 128)):
                        ps = psX.tile([128, 512], F32, tag="x", name="trps")
                        tp = ps.bitcast(BF16)
                        nc.tensor.transpose(
                            tp[:, 0:128], VT[:, g * 128:(g + 1) * 128], id_t[:]
                        )
                        base = g * VSTRIDE
                        nc.vector.tensor_copy(
                            Vaug[:, base:base + DH], tp[:, 0:DH]
                        )
                        nc.vector.tensor_copy(
                            Vaug[:, base + DH + 1:base + 2 * DH + 1],
                            tp[:, DH:2 * DH],
                        )

                def emit_scale(b, j, spool):
                    # selector matmul broadcasts Z: ps[i, q] = Z_{head i//64}[q]
                    # reciprocal in the 128-lane broadcast domain, then one
                    # DVE multiply normalizes attnU columns in place
                    ps = psX.tile([128, 512], F32, tag="x", name="scaleps")
                    jsl = slice(j * 512, (j + 1) * 512)
                    nc.tensor.matmul(ps[:], sel_t[:], zb[b][:, jsl],
                                     start=True, stop=True)
                    ss = spool.tile([128, 512], F32, tag="ss", name="sstile")
                    nc.vector.reciprocal(ss[:], ps[:])
                    nc.vector.tensor_tensor(
                        attnU[b][:, jsl], attnU[b][:, jsl], ss[:],
                        op=ALU.mult,
                    )

                def emit_cu(b, rc, oc, opool):
                    # one output-projection unit: 128 tokens x 512 channels,
                    # single full-contraction matmul on normalized attnU
                    lrsl = slice(rc * 128, (rc + 1) * 128)
                    osl = slice(oc * 512, (oc + 1) * 512)
                    ps = psX.tile([128, 512], F32, tag="x", name="cups")
                    nc.tensor.matmul(ps[:], attnU[b][:, lrsl], wo_t[:, osl],
                                     start=True, stop=True)
                    st = opool.tile([128, 512], BF16, tag="cu", name="cust")
                    nc.vector.tensor_copy(st[:], ps[:])
                    nc.sync.dma_start(out_d[b * L + rc * 128:
                                            b * L + (rc + 1) * 128, osl], st[:])

                with (
                    tc.tile_pool(name="xin", bufs=2) as xpool,
                    tc.tile_pool(name="scaleP", bufs=2) as spool,
                    tc.tile_pool(name="expP", bufs=3) as epool,
                    tc.tile_pool(name="a65P", bufs=2) as apool,
                    tc.tile_pool(name="oout", bufs=3) as opool,
                    tc.tile_pool(name="psS", bufs=2, space="PSUM") as psS,
                    tc.tile_pool(name="psPV", bufs=1, space="PSUM") as psPV,
                ):
                    # ---- A0: batch-0 QKV ----
                    emit_chunk_dma(0, xpool)
                    emit_chunk_dma(1, xpool)
                    for c in range(4):
                        emit_proj(c, wq_t, bq_t, QT)
                        emit_proj(c, wk_t, bk_t, KT)
                        if c + 2 < 4:
                            emit_chunk_dma(c + 2, xpool)
                        emit_proj(c, wv_t, bv_t, VT)
                        emit_tr(c)

                    # ---- side-work queues for phase B's PE slack ----
                    # a1: batch-1 QKV (1 item per 4 kt slots)
                    # pc: batch-0 scale + output projection (1 item per slot)
                    a1 = [lambda: emit_chunk_dma(4, xpool)]
                    for c in range(4, NCH):
                        a1.append(lambda c=c: emit_proj(c, wq_t, bq_t, QT))

                        def k_and_dma(c=c):
                            emit_proj(c, wk_t, bk_t, KT)
                            if c + 1 < NCH:
                                emit_chunk_dma(c + 1, xpool)
                        a1.append(k_and_dma)
                        a1.append(lambda c=c: emit_proj(c, wv_t, bv_t, VT))
                        a1.append(lambda c=c: emit_tr(c))
                    pc = []

                    slot = [0]

                    def pump():
                        s = slot[0]
                        slot[0] += 1
                        if a1:
                            if s % 4 == 0:
                                a1.pop(0)()
                            return
                        if pc:
                            pc.pop(0)()

                    # ---- B: attention, ACT-paced; side work fills the PE ----
                    for b in range(B):
                        for h in range(2):
                            hs = slice(h * 64, (h + 1) * 64)
                            for qc in range(NQC):
                                q0 = b * L + qc * QC
                                pv = psPV.tile([65, QC], F32, tag="pv")
                                for kt in range(NKB):
                                    ksl = slice(b * L + kt * 128,
                                                b * L + (kt + 1) * 128)
                                    sc = psS.tile([128, QC], F32, tag="sc")
                                    for hf in range(QC // 512):
                                        nc.tensor.matmul(
                                            sc[:, hf * 512:(hf + 1) * 512],
                                            KT[hs, ksl],
                                            QT[hs, q0 + hf * 512:
                                               q0 + hf * 512 + 512],
                                            start=True, stop=True,
                                        )
                                    ex = epool.tile([128, QC], BF16, tag="ex")
                                    nc.scalar.activation(ex[:], sc[:], AF.Exp, scale=0.125)
                                    g = b * NKB + kt
                                    vb = g * VSTRIDE + h * (DH + 1)
                                    for hf in range(QC // 512):
                                        nc.tensor.matmul(
                                            pv[:, hf * 512:(hf + 1) * 512],
                                            Vaug[:, vb:vb + DH + 1],
                                            ex[:, hf * 512:(hf + 1) * 512],
                                            start=(kt == 0),
                                            stop=(kt == NKB - 1),
                                        )
                                    pump()
                                # evacuate: one copy frees the accumulator
                                a65 = apool.tile([65, QC], BF16, tag="a65")
                                nc.vector.tensor_copy(a65[:], pv[0:65, :])
                                lqsl = slice(qc * QC, (qc + 1) * QC)
                                nc.vector.tensor_copy(
                                    attnU[b][h * 64:(h + 1) * 64, lqsl],
                                    a65[0:64, :],
                                )
                                nc.gpsimd.tensor_copy(
                                    zb[b][32 * h:32 * h + 1, lqsl],
                                    a65[64:65, :],
                                )
                            if b == 0 and h == 1:
                                # batch-0 attention done: queue its
                                # normalization + output projection
                                for j in range(4):
                                    pc.append(lambda j=j:
                                              emit_scale(0, j, spool))
                                    pc.extend(
                                        [lambda rc=rc, oc=oc:
                                         emit_cu(0, rc, oc, opool)
                                         for rc in range(4 * j, 4 * j + 4)
                                         for oc in range(2)]
                                    )

                    # drain whatever the slots didn't absorb
                    while a1:
                        a1.pop(0)()
                    while pc:
                        pc.pop(0)()

                # ---- tail: batch-1 normalize + output projection ----
                with (
                    tc.tile_pool(name="oout2", bufs=4) as opool2,
                    tc.tile_pool(name="scaleP2", bufs=2) as spool2,
                    tc.tile_pool(name="psD", bufs=3, space="PSUM") as psD,
                ):
                    def emit_cu_tail(rc, alt):
                        lrsl = slice((rc - 16) * 128, (rc - 15) * 128)
                        ps = psD.tile([128, 1024], F32, tag="d", name="cutps")
                        for oc in range(2):
                            nc.tensor.matmul(
                                ps[:, oc * 512:(oc + 1) * 512],
                                attnU[1][:, lrsl],
                                wo_t[:, oc * 512:(oc + 1) * 512],
                                start=True, stop=True,
                            )
                        st = opool2.tile([128, 1024], BF16, tag="st",
                                         name="cutst")
                        if alt:
                            nc.scalar.activation(st[:], ps[:], AF.Copy)
                        else:
                            nc.vector.tensor_copy(st[:], ps[:])
                        nc.sync.dma_start(
                            out_d[L + (rc - 16) * 128:
                                  L + (rc - 15) * 128, :], st[:]
                        )

                    emit_scale(1, 0, spool2)
                    emit_scale(1, 1, spool2)
                    for rc in range(16, 24):
                        if rc == 20:
                            emit_scale(1, 2, spool2)
                            emit_scale(1, 3, spool2)
                        emit_cu_tail(rc, alt=True)
                    for rc in range(24, 32):
                        emit_cu_tail(rc, alt=True)

    nc.compile()
    _NC_CACHE["nc"] = nc
    return nc


def _shard_inputs(x, W_qkv, b_qkv, W_o):
    import ml_dtypes
    BF = ml_dtypes.bfloat16
    xT = np.ascontiguousarray(
        x.reshape(BL, D_MODEL).T.astype(BF)
    )
    ident = np.eye(128, dtype=BF)

    def lhsT_layout(w):
        # [D_MODEL, 128] -> [128, NKT*128] with [p, kt*128+ch] = w[kt*128+p, ch]
        return np.ascontiguousarray(
            w.reshape(NKT, 128, 128).transpose(1, 0, 2)
            .reshape(128, NKT * 128).astype(BF)
        )

    in_maps = []
    for c in range(NCORES):
        cs = slice(c * 128, (c + 1) * 128)
        wq = W_qkv[:, cs]
        wk = W_qkv[:, D_MODEL:][:, cs]
        wv = W_qkv[:, 2 * D_MODEL:][:, cs]
        in_maps.append({
            "xT": xT,
            "wq": lhsT_layout(wq), "wk": lhsT_layout(wk),
            "wv": lhsT_layout(wv),
            "bq": np.ascontiguousarray(
                b_qkv[cs], dtype=np.float32).reshape(128, 1),
            "bk": np.ascontiguousarray(
                b_qkv[D_MODEL:][cs], dtype=np.float32).reshape(128, 1),
            "bv": np.ascontiguousarray(
                b_qkv[2 * D_MODEL:][cs], dtype=np.float32).reshape(128, 1),
            "wo": np.ascontiguousarray(W_o[cs, :].astype(BF)),
            "ident": ident,
        })
    return in_maps


def _run(inputs, trace=False, tmpdir=None):
    from concourse.bass_utils import run_bass_kernel_spmd

    _register_ntff_hook()
    nc = _build()
    in_maps = _shard_inputs(
        np.asarray(inputs["x"], dtype=np.float32),
        np.asarray(inputs["W_qkv"], dtype=np.float32),
        np.asarray(inputs["b_qkv"], dtype=np.float32),
        np.asarray(inputs["W_o"], dtype=np.float32),
    )
    res = run_bass_kernel_spmd(nc, in_maps, core_ids=list(range(NCORES)),
                               trace=trace, tmpdir=tmpdir)
    partial = np.zeros((BL, D_MODEL), dtype=np.float64)
    for c in range(NCORES):
        partial += res.results[c]["out"].astype(np.float64)
    out = (partial + np.asarray(inputs["b_o"], dtype=np.float64)).astype(np.float32)
    return out.reshape(B, L, D_MODEL), res


def kernel(**inputs) -> np.ndarray:
    out, _ = _run(inputs, trace=False)
    return out
